# revision 1
# baseline (speedup 1.0000x reference)
"""Distributed decoder-layer kernel for TRN2 (8 NeuronCores), v3.

Sharding: core i -> batch b = i//4, tp = i%4 (data parallel on B, tensor
parallel over the 16 heads).
  - Two 8-core AllToAlls (one per head-pair) swap the head-split for a
    sequence-split; the out-projection then runs locally against a
    per-core zero-padded wo (cross-batch-group rows zeroed by the host),
    which absorbs the group-dependence the SPMD program cannot express.
    A2A#0 fires after heads 0-1 and hides under heads 2-3; the
    out-projection's pair-0 half runs in 8 open PSUM accumulators while
    A2A#1 is still in flight.
  - LN gamma/beta are folded into wq/wk/wv/w1 (and biases) on the host, so
    the kernel's layernorm is pure normalize; the transpose to feature-
    major runs on the DMA xbar (dma_start_transpose), not the PE.
    Channel ordering within hT/h2nT is e = p*8 + a (partition-major), so
    the host lays out wq/wk/wv/w1 rows with a plain reshape(128, 8, ...).
  - Attention is software-pipelined: score matmuls run LOOK=3 tiles ahead
    of the A@V accumulation so the in-order PE queue never stalls on the
    exp() activation, keeping the PE HAM-warm.
  - LN2 / FFN run sequence-parallel on the own 512 rows with full weights
    (w1 and w2 streamed from HBM in bf16, once each).

Softmax needs no max-subtraction (logits are O(1)); row sums come free from
a trailing ones-column in V; 1/sum is broadcast across partitions with a
K=1 matmul outer product, batched one tile behind the main stream.

x arrives in bf16 (host-cast) to halve the startup DMA; the residual x
slice (xr) stays f32 for output accuracy. Weight DMAs ride the scalar
HWDGE queue (the big wo load is emitted mid-attention) so the x stream on
the sync queue is never blocked.
"""

import numpy as np

import concourse.bass as bass
import concourse.mybir as mybir
import concourse.tile as tile
from concourse import bacc

F32 = mybir.dt.float32
BF16 = mybir.dt.bfloat16
AF = mybir.ActivationFunctionType
OP = mybir.AluOpType

N_CORES = 8
P = 128
S = 2048          # sequence length
E = 1024          # embed
FF = 4096         # mlp hidden
HL = 4            # local heads
DH = 64           # head dim
CH = HL * DH      # local channels = 256
R = S // 4        # rows per core owned after AllToAll = 512
ET = E // P       # 8 e-tiles
ST = S // P       # 16 s-tiles
FT = FF // P      # 32 f-tiles
RT = R // P       # 4 own-row tiles
EPS = 1e-5
LOOK = 4          # attention score->AV software-pipeline depth
A2A_GROUP = [[0, 1, 2, 3, 4, 5, 6, 7]]
USE_COLLECTIVE = True
FFN_ACT = AF.Gelu  # swapped to Tanh for simulator runs


def build_nc():
    nc = bacc.Bacc("TRN2", target_bir_lowering=False, debug=False,
                   num_devices=N_CORES)

    # ---- DRAM parameters (per-core shards; host does slicing/casting) ----
    x_b = nc.declare_dram_parameter("x_b", [S, E], BF16, isOutput=False)
    xr_f = nc.declare_dram_parameter("xr_f", [R, E], F32, isOutput=False)
    # wq|wk|wv (gamma1-folded, q pre-scaled)
    wqkv_b = nc.declare_dram_parameter("wqkv_b", [E, 3 * CH], BF16,
                                       isOutput=False)
    bq_f = nc.declare_dram_parameter("bq_f", [P, 2], F32, isOutput=False)
    bk_f = nc.declare_dram_parameter("bk_f", [P, 2], F32, isOutput=False)
    bv_b = nc.declare_dram_parameter("bv_b", [CH], BF16, isOutput=False)
    # wo expanded to 2048 rows: row i*256+d*128+p = wo[(i%4)*256+d*128+p]
    # for cores i in this core's batch group, zeros for the other group.
    wo_b = nc.declare_dram_parameter("wo_b", [2 * E, E], BF16, isOutput=False)
    bo_b = nc.declare_dram_parameter("bo_b", [E], BF16, isOutput=False)
    # gamma2-folded
    w1_b = nc.declare_dram_parameter("w1_b", [E, FF], BF16, isOutput=False)
    b1_f = nc.declare_dram_parameter("b1_f", [P, FT], F32, isOutput=False)
    w2_b = nc.declare_dram_parameter("w2_b", [FF, E], BF16, isOutput=False)
    b2_b = nc.declare_dram_parameter("b2_b", [E], BF16, isOutput=False)
    out_p = nc.declare_dram_parameter("out", [R, E], F32, isOutput=True)

    with tile.TileContext(nc) as tc:
        _emit(nc, tc, locals())
    nc.compile()
    return nc


def _emit(nc, tc, t):
    x_b, xr_f = t["x_b"], t["xr_f"]
    wqkv_b = t["wqkv_b"]
    bq_f, bk_f, bv_b = t["bq_f"], t["bk_f"], t["bv_b"]
    wo_b, bo_b = t["wo_b"], t["bo_b"]
    w1_b, b1_f, w2_b, b2_b = t["w1_b"], t["b1_f"], t["w2_b"], t["b2_b"]
    out_p = t["out_p"]

    const = tc.alloc_tile_pool(name="const", bufs=1)
    wpool = tc.alloc_tile_pool(name="wpool", bufs=1)
    act = tc.alloc_tile_pool(name="act", bufs=1)
    xpool = tc.alloc_tile_pool(name="xpool", bufs=3)
    xnpool = tc.alloc_tile_pool(name="xnpool", bufs=2)
    scrpool = tc.alloc_tile_pool(name="scrpool", bufs=1)
    stats = tc.alloc_tile_pool(name="stats", bufs=6)
    eppool = tc.alloc_tile_pool(name="eppool", bufs=5)
    afpool = tc.alloc_tile_pool(name="afpool", bufs=4)
    recpool = tc.alloc_tile_pool(name="recpool", bufs=4)
    ystpool = tc.alloc_tile_pool(name="ystpool", bufs=3)
    xrpool = tc.alloc_tile_pool(name="xrpool", bufs=2)
    w1pool = tc.alloc_tile_pool(name="w1pool", bufs=2)
    w2pool = tc.alloc_tile_pool(name="w2pool", bufs=2)
    opool = tc.alloc_tile_pool(name="opool", bufs=2)
    dram = tc.alloc_tile_pool(name="dram", bufs=1, space="DRAM")
    pmm = tc.alloc_tile_pool(name="pmm", bufs=4, space="PSUM")
    pacc = tc.alloc_tile_pool(name="pacc", bufs=2, space="PSUM")
    pnew = tc.alloc_tile_pool(name="pnew", bufs=2, space="PSUM")

    # ---- small constants + QKV weights (scalar HWDGE queue; the sync
    # queue carries the x stream so neither blocks the other) ----
    bqc = const.tile([P, 2], F32, name="bqc")
    nc.scalar.dma_start(out=bqc[:], in_=bq_f[:, :])
    bkc = const.tile([P, 2], F32, name="bkc")
    nc.scalar.dma_start(out=bkc[:], in_=bk_f[:, :])
    bvr = const.tile([1, CH], BF16, name="bvr")
    nc.scalar.dma_start(out=bvr[:], in_=bv_b.rearrange("(a f) -> a f", a=1))
    wqkv_sb = wpool.tile([P, ET, 3 * CH], BF16, name="wqkv_sb")
    nc.scalar.dma_start(out=wqkv_sb[:],
                        in_=wqkv_b.rearrange("(a p) c -> p a c", p=P))
    b1c = const.tile([P, FT], F32, name="b1c")
    nc.scalar.dma_start(out=b1c[:], in_=b1_f[:, :])
    bor = const.tile([1, E], BF16, name="bor")
    nc.scalar.dma_start(out=bor[:], in_=bo_b.rearrange("(a f) -> a f", a=1))
    b2r = const.tile([1, E], BF16, name="b2r")
    nc.scalar.dma_start(out=b2r[:], in_=b2_b.rearrange("(a f) -> a f", a=1))

    # ---- computed constants ----
    # mbig[p, j] = 1 if (j - 384 - p) >= 0 else 0  -> causal mask slices
    mbig = const.tile([P, 896], BF16, name="mbig")
    nc.vector.memset(mbig[:], 1.0)
    nc.gpsimd.affine_select(
        out=mbig[:], in_=mbig[:], compare_op=OP.is_ge, fill=0.0,
        base=-384, channel_multiplier=-1, pattern=[[1, 896]],
    )
    ones_row = const.tile([1, P], BF16, name="ones_row")
    nc.vector.memset(ones_row[:], 1.0)
    ones65 = const.tile([P, DH], BF16, name="ones65")
    nc.vector.memset(ones65[:], 1.0)
    eps_c = const.tile([P, 1], F32, name="eps_c")
    nc.vector.memset(eps_c[:], EPS)

    # ---- persistent activations ----
    # hT and gT share one 32KB/partition slot (hT dies after QKV); Y2T
    # shares it too (written only after hT's last read, dead before gT).
    hT = act.tile([P, ET, S], BF16, tag="big", name="hT")
    QT = act.tile([P, 2, S], BF16, name="QT")      # 2 head-pairs packed
    # KT and h2nT share a slot (KT dies after attention).
    KT = act.tile([P, 2, S], BF16, tag="mid", name="KT")
    # v with a trailing ones column (col 64) so A@V also yields sum(exp)
    # in psum row 64 (row offsets must stay 32-aligned for engine access)
    V_sb = act.tile([P, ST, HL, DH + 1], BF16, name="V_sb")
    Y2T = act.tile([P, ET, S], BF16, tag="big", name="Y2T")
    h2_sb = act.tile([P, RT, E], F32, name="h2_sb")
    yf = [act.tile([P, 8, R], BF16, name=f"yf{d}") for d in range(2)]

    nc.vector.memset(V_sb[:, :, :, DH:DH + 1], 1.0)

    # ================= layernorm helper (pure normalize; gamma/beta are
    # folded into the consuming weights host-side) =================
    def layernorm_tile(src_ap, dstT, st):
        s_sum = stats.tile([P, 1], F32, tag="s_sum", name="s_sum")
        nc.vector.reduce_sum(s_sum[:], src_ap, axis=mybir.AxisListType.X)
        s_mean = stats.tile([P, 1], F32, tag="s_mean", name="s_mean")
        nc.scalar.mul(s_mean[:], s_sum[:], 1.0 / E)
        scr_t = scrpool.tile([P, E], BF16, name="scr_t")
        s_msq = stats.tile([P, 1], F32, tag="s_msq", name="s_msq")
        nc.scalar.activation(scr_t[:], src_ap, AF.Square, scale=1.0 / 32.0,
                             accum_out=s_msq[:])
        s_mu2 = stats.tile([P, 1], F32, tag="s_mu2", name="s_mu2")
        nc.scalar.square(s_mu2[:], s_mean[:])
        s_var = stats.tile([P, 1], F32, tag="s_var", name="s_var")
        nc.vector.tensor_scalar(s_var[:], s_msq[:], s_mu2[:], None,
                                op0=OP.subtract)
        s_std = stats.tile([P, 1], F32, tag="s_std", name="s_std")
        nc.scalar.activation(s_std[:], s_var[:], AF.Sqrt, bias=eps_c[:])
        s_rstd = stats.tile([P, 1], F32, tag="s_rstd", name="s_rstd")
        nc.vector.reciprocal(s_rstd[:], s_std[:])
        xn_t = xnpool.tile([P, E], BF16, name="xn_t")
        nc.vector.tensor_scalar(xn_t[:], src_ap, s_mean[:], s_rstd[:],
                                op0=OP.subtract, op1=OP.mult)
        # feature-major via the DMA xbar; channel e lands at (a=e//128, p=e%128)
        nc.sync.dma_start_transpose(
            out=dstT[:, :, st * P:(st + 1) * P], in_=xn_t[:])

    # ================= LN1 + QKV + V, interleaved in groups of 4 s-tiles
    # so QKV matmuls start as soon as their hT columns exist ============
    def qkv_proj(qc):
        qs = slice(qc * 512, (qc + 1) * 512)
        for d in range(2):        # head pair (2 heads = 128 channels)
            ps = pmm.tile([P, 512], F32, tag="mm", name="ps_q")
            for et in range(ET):
                nc.tensor.matmul(ps[:], lhsT=wqkv_sb[:, et, d * P:(d + 1) * P],
                                 rhs=hT[:, et, qs],
                                 start=(et == 0), stop=(et == ET - 1))
            # Q evac on DVE, K evac on ACT (balance the two engines)
            nc.vector.tensor_scalar(QT[:, d, qs], ps[:], bqc[:, d:d + 1],
                                    None, op0=OP.add)
            ps = pmm.tile([P, 512], F32, tag="mm", name="ps_k")
            for et in range(ET):
                nc.tensor.matmul(ps[:],
                                 lhsT=wqkv_sb[:, et, CH + d * P:CH + (d + 1) * P],
                                 rhs=hT[:, et, qs],
                                 start=(et == 0), stop=(et == ET - 1))
            nc.scalar.activation(KT[:, d, qs], ps[:], AF.Identity,
                                 bias=bkc[:, d:d + 1])

    def v_proj(st):
        ps = pmm.tile([P, 512], F32, tag="mm", name="ps_v")
        for et in range(ET):
            nc.tensor.matmul(ps[:, 0:CH],
                             lhsT=hT[:, et, st * P:(st + 1) * P],
                             rhs=wqkv_sb[:, et, 2 * CH:3 * CH],
                             start=(et == 0), stop=False)
        nc.tensor.matmul(ps[:, 0:CH], lhsT=ones_row[0:1, 0:P],
                         rhs=bvr[0:1, :], start=False, stop=True)
        nc.vector.tensor_copy(
            V_sb[:, st, :, 0:DH],
            ps[:, 0:CH].rearrange("p (h d) -> p h d", h=HL))

    for g in range(4):
        for st in range(4 * g, 4 * g + 4):
            xt_t = xpool.tile([P, E], BF16, name="xt_t")
            nc.sync.dma_start(out=xt_t[:], in_=x_b[st * P:(st + 1) * P, :])
            layernorm_tile(xt_t[:], hT, st)
        qkv_proj(g)
        for st in range(4 * g, 4 * g + 4):
            v_proj(st)

    # ================= attention =================
    def flush_norm(pending):
        # batched tail: broadcast 1/sum across partitions (K=1 matmul) and
        # normalize -- runs one tile behind the main stream so the bc
        # matmul never waits on the reciprocal
        for h, qc, afb, recb in pending:
            d, po = h // 2, (h % 2) * DH
            qs = slice(qc * 512, (qc + 1) * 512)
            bc = pmm.tile([P, 512], F32, tag="mm", name="bc")
            nc.tensor.matmul(bc[0:DH, :], lhsT=ones65[DH:DH + 1, :],
                             rhs=recb[DH:DH + 1, :], start=True, stop=True)
            yst = ystpool.tile([DH, 512], BF16, name="yst")
            nc.vector.tensor_mul(yst[:], afb[:], bc[0:DH, :])
            # pack into Y2T at channel offset (partition shift via SB2SB DMA)
            nc.sync.dma_start(out=Y2T[po:po + DH, d, qs], in_=yst[:])
        pending.clear()

    # A2A buffers: ag_in_d[j, p, s] = head-pair d, dest-quarter j%4's Y^T
    ag_in = [dram.tile([8, P, 512], BF16, name=f"ag_in{d}") for d in range(2)]
    ag_out = [dram.tile([8, P, 512], BF16, name=f"ag_out{d}")
              for d in range(2)]

    def send_pair(d):
        for half in range(2):
            nc.sync.dma_start(
                out=ag_in[d][half * 4:(half + 1) * 4].rearrange(
                    "q p s -> p q s"),
                in_=Y2T[:, d, :].rearrange("p (q s) -> p q s", q=4))
        if USE_COLLECTIVE:
            nc.gpsimd.collective_compute(
                "AllToAll", OP.bypass, replica_groups=A2A_GROUP,
                ins=[ag_in[d].opt()], outs=[ag_out[d].opt()])
        else:
            nc.sync.dma_start(out=ag_out[d][:, :, :], in_=ag_in[d][:, :, :])

    pending = []
    for h in range(HL):
        d, po = h // 2, (h % 2) * DH
        for qc in (1, 2, 3, 0):
            if len(pending) > 1:
                flush_norm(pending[:-1])
                pending[:] = pending[-1:]
            qs = slice(qc * 512, (qc + 1) * 512)
            acc = pacc.tile([P, 512], F32, tag="acc", name="acc")
            nk = (qc + 1) * 4
            eps_q = []

            def emit_av(kt2, ep2):
                nc.tensor.matmul(acc[0:DH + 1, :],
                                 lhsT=V_sb[:, kt2, h, :], rhs=ep2[:],
                                 start=(kt2 == 0), stop=(kt2 == nk - 1))

            for kt in range(nk):
                ps = pmm.tile([P, 512], F32, tag="mm", name="ps_s")
                nc.tensor.matmul(ps[:],
                                 lhsT=KT[po:po + DH, d, kt * P:(kt + 1) * P],
                                 rhs=QT[po:po + DH, d, qs],
                                 start=True, stop=True)
                ep = eppool.tile([P, 512], BF16, name="ep")
                nc.scalar.activation(ep[:], ps[:], AF.Exp)
                if kt >= qc * 4:
                    r_off = kt * P - qc * 512
                    nc.vector.tensor_mul(
                        ep[:], ep[:], mbig[:, 384 - r_off: 896 - r_off])
                eps_q.append(ep)
                if kt >= LOOK:
                    emit_av(kt - LOOK, eps_q[kt - LOOK])
            for kt in range(max(0, nk - LOOK), nk):
                emit_av(kt, eps_q[kt])
            # rows 0..63 of acc = unnormalized attn out^T; row 64 = sum(exp)
            afb = afpool.tile([DH, 512], BF16, name="afb")
            nc.scalar.copy(afb[:], acc[0:DH, :])
            recf = recpool.tile([DH + 1, 512], F32, tag="recf", name="recf",
                                bufs=2)
            nc.vector.reciprocal(recf[DH:DH + 1, :], acc[DH:DH + 1, :])
            recb = recpool.tile([DH + 1, 512], BF16, tag="recb", name="recb")
            nc.vector.tensor_copy(recb[DH:DH + 1, :], recf[DH:DH + 1, :])
            pending.append((h, qc, afb, recb))
        if h == 1:
            flush_norm(pending)
            send_pair(0)   # overlaps heads 2-3
            # 4MB wo_expanded load: emitted here so its DMA neither blocks
            # the startup x stream nor collides with the early weights
            wo_sb = wpool.tile([P, 2 * ET, E], BF16, name="wo_sb")
            nc.scalar.dma_start(
                out=wo_sb[:], in_=wo_b.rearrange("(a p) e -> p a e", p=P))
        elif h == 3:
            flush_norm(pending)
            send_pair(1)

    # yf_d[p, i, s] = core i's pair-d channels for own rows (junk rows are
    # killed by the zero rows of wo_expanded). yf[1]'s read is emitted
    # AFTER the phase-A matmuls: Tile's DMA sem lanes are shared, and an
    # earlier emission would fold A2A#1 into phase A's wait threshold.
    nc.sync.dma_start(out=yf[0][:], in_=ag_out[0].rearrange("i p s -> p i s"))
    # residual rows ride the scalar queue (sync is blocked on A2A#1)
    xr_tiles = []
    for st in range(RT):
        xr_t = xrpool.tile([P, E], F32, name="xr_t")
        nc.scalar.dma_start(out=xr_t[:], in_=xr_f[st * P:(st + 1) * P, :])
        xr_tiles.append(xr_t)

    # ========== out-projection: phase A (pair-0 channels) fills 8 open
    # PSUM accumulators while A2A#1 is in flight; phase B adds pair-1,
    # bias, and the residual ==========
    def oacc_tile(idx):
        if idx < 4:
            return pmm.tile([P, 512], F32, tag="mm", name=f"oacc{idx}")
        if idx < 6:
            return pacc.tile([P, 512], F32, tag="acc", name=f"oacc{idx}")
        return pnew.tile([P, 512], F32, tag="o2", name=f"oacc{idx}")

    oaccs = []
    for st in range(RT):
        for ec in range(2):
            es = slice(ec * 512, (ec + 1) * 512)
            ps = oacc_tile(st * 2 + ec)
            oaccs.append(ps)
            for i in range(8):
                nc.tensor.matmul(ps[:], lhsT=yf[0][:, i, st * P:(st + 1) * P],
                                 rhs=wo_sb[:, 2 * i, es],
                                 start=(i == 0), stop=False)
    nc.sync.dma_start(out=yf[1][:], in_=ag_out[1].rearrange("i p s -> p i s"))
    for st in range(RT):
        for ec in range(2):
            es = slice(ec * 512, (ec + 1) * 512)
            ps = oaccs[st * 2 + ec]
            for i in range(8):
                nc.tensor.matmul(ps[:], lhsT=yf[1][:, i, st * P:(st + 1) * P],
                                 rhs=wo_sb[:, 2 * i + 1, es],
                                 start=False, stop=False)
            nc.tensor.matmul(ps[:], lhsT=ones_row[0:1, 0:P],
                             rhs=bor[0:1, es], start=False, stop=True)
            nc.vector.tensor_add(h2_sb[:, st, es], ps[:],
                                 xr_tiles[st][:, es])

    # ================= LN2 =================
    h2nT = act.tile([P, ET, R], BF16, tag="mid", name="h2nT")
    for st in range(RT):
        layernorm_tile(h2_sb[:, st, :], h2nT, st)

    # ================= FFN1 (gelu) =================
    gT = act.tile([P, FT, R], BF16, tag="big", name="gT")
    for fc in range(16):
        w1_t = w1pool.tile([P, ET, 256], BF16, name="w1_t")
        nc.scalar.dma_start(
            out=w1_t[:],
            in_=w1_b[:, fc * 256:(fc + 1) * 256].rearrange(
                "(a p) f -> p a f", p=P))
        for ft in range(2):
            ftg = fc * 2 + ft
            ps = pmm.tile([P, 512], F32, tag="mm", name="ps_f1")
            for et in range(ET):
                nc.tensor.matmul(ps[:],
                                 lhsT=w1_t[:, et, ft * P:(ft + 1) * P],
                                 rhs=h2nT[:, et, :],
                                 start=(et == 0), stop=(et == ET - 1))
            nc.scalar.activation(gT[:, ftg, :], ps[:], FFN_ACT,
                                 bias=b1c[:, ftg:ftg + 1])

    # ================= FFN2 + residual =================
    # Free the small psum pools; FFN2 wants 4 x [128, 1024] accumulators.
    pnew.release()
    pacc.release()
    pmm.release()
    pffn = tc.alloc_tile_pool(name="pffn", bufs=1, space="PSUM")
    accs = [pffn.tile([P, E], F32, name=f"facc{st}") for st in range(RT)]
    for ftg in range(FT):
        w2_t = w2pool.tile([P, E], BF16, name="w2_t")
        nc.scalar.dma_start(out=w2_t[:], in_=w2_b[ftg * P:(ftg + 1) * P, :])
        for st in range(RT):
            for ec in range(2):
                es = slice(ec * 512, (ec + 1) * 512)
                nc.tensor.matmul(accs[st][:, es],
                                 lhsT=gT[:, ftg, st * P:(st + 1) * P],
                                 rhs=w2_t[:, es],
                                 start=(ftg == 0), stop=False)
    for st in range(RT):
        for ec in range(2):
            es = slice(ec * 512, (ec + 1) * 512)
            nc.tensor.matmul(accs[st][:, es], lhsT=ones_row[0:1, 0:P],
                             rhs=b2r[0:1, es], start=False, stop=True)
            o_t = opool.tile([P, 512], F32, name="o_t")
            nc.vector.tensor_add(o_t[:], accs[st][:, es], h2_sb[:, st, es])
            nc.sync.dma_start(out=out_p[st * P:(st + 1) * P, es], in_=o_t[:])

    for pool in (pffn, dram, opool, w2pool, w1pool, xrpool, ystpool,
                 recpool, afpool, eppool, stats, scrpool, xnpool, xpool, act,
                 wpool, const):
        pool.release()


_NC_CACHE = None


def _get_nc():
    global _NC_CACHE
    if _NC_CACHE is None:
        _NC_CACHE = build_nc()
    return _NC_CACHE


def kernel(**inputs):
    import ml_dtypes
    bf = ml_dtypes.bfloat16
    nc = _get_nc()

    x = np.asarray(inputs["x"], np.float32)
    wq = np.asarray(inputs["wq"], np.float32)
    wk = np.asarray(inputs["wk"], np.float32)
    wv = np.asarray(inputs["wv"], np.float32)
    wo = np.asarray(inputs["wo"], np.float32)
    w1 = np.asarray(inputs["w1"], np.float32)
    w2 = np.asarray(inputs["w2"], np.float32)
    bq = np.asarray(inputs["bq"], np.float32)
    bk = np.asarray(inputs["bk"], np.float32)
    bv = np.asarray(inputs["bv"], np.float32)
    bo = np.asarray(inputs["bo"], np.float32)
    b1 = np.asarray(inputs["b1"], np.float32)
    b2 = np.asarray(inputs["b2"], np.float32)
    ln1g = np.asarray(inputs["ln1_g"], np.float32)
    ln1b = np.asarray(inputs["ln1_b"], np.float32)
    ln2g = np.asarray(inputs["ln2_g"], np.float32)
    ln2b = np.asarray(inputs["ln2_b"], np.float32)

    sc = 1.0 / np.sqrt(DH)
    # fold LN1 gamma into wq/wk/wv rows and LN1 beta into the biases;
    # same for LN2 gamma/beta into w1/b1. Kernel LN is pure normalize.
    wq_f = ln1g[:, None] * wq * sc
    wk_f = ln1g[:, None] * wk
    wv_f = ln1g[:, None] * wv
    bq_fold = bq * sc + ln1b @ (wq * sc)
    bk_fold = bk + ln1b @ wk
    bv_fold = bv + ln1b @ wv
    w1_f = ln2g[:, None] * w1
    b1_fold = b1 + ln2b @ w1

    in_maps = []
    for core in range(N_CORES):
        b, tp = core // 4, core % 4
        c0 = tp * CH
        wqkv = np.concatenate(
            [wq_f[:, c0:c0 + CH], wk_f[:, c0:c0 + CH], wv_f[:, c0:c0 + CH]],
            axis=1)
        # wo expanded: rows i*256+c hold wo[(i-4b)*256+c] for cores i in
        # this core's batch group, zeros for the other group's rows
        woe = np.zeros((N_CORES, CH, E), np.float32)
        for r in range(4):
            woe[4 * b + r] = wo[r * CH:(r + 1) * CH]
        in_maps.append({
            "x_b": np.ascontiguousarray(x[b]).astype(bf),
            "xr_f": np.ascontiguousarray(x[b, tp * R:(tp + 1) * R]),
            "wqkv_b": np.ascontiguousarray(wqkv).astype(bf),
            "bq_f": np.ascontiguousarray(
                bq_fold[c0:c0 + CH].reshape(2, P).T),
            "bk_f": np.ascontiguousarray(
                bk_fold[c0:c0 + CH].reshape(2, P).T),
            "bv_b": np.ascontiguousarray(bv_fold[c0:c0 + CH]).astype(bf),
            "wo_b": np.ascontiguousarray(woe.reshape(2 * E, E)).astype(bf),
            "bo_b": bo.astype(bf),
            "w1_b": np.ascontiguousarray(w1_f).astype(bf),
            "b1_f": np.ascontiguousarray(b1_fold.reshape(FT, P).T),
            "w2_b": w2.astype(bf), "b2_b": b2.astype(bf),
        })

    from concourse.bass_utils import run_bass_kernel_spmd
    import os
    kw = {}
    if os.environ.get("BASS_TRACE"):
        kw = dict(trace=True, trace_cores=list(range(N_CORES)))
    res = run_bass_kernel_spmd(nc, in_maps, core_ids=list(range(N_CORES)), **kw)
    if res.exec_time_ns is not None:
        print(f"HW exec time: {res.exec_time_ns} ns")
        print(f"HW exec time mean: {res.mean_exec_time_ns} ns")

    out = np.empty((2, S, E), np.float32)
    for core in range(N_CORES):
        b, tp = core // 4, core % 4
        out[b, tp * R:(tp + 1) * R] = res.results[core]["out"]
    return out



# revision 21
# speedup vs baseline: 1.1073x; 1.1073x over previous
"""Distributed decoder-layer kernel for TRN2 (8 NeuronCores), v4.

Sharding: core i -> batch b = i//4, tp = i%4 (data parallel on B, tensor
parallel over the 16 heads).
  - Two 8-core AllToAlls (one per head-pair) swap the head-split for a
    sequence-split; the out-projection then runs locally against a
    per-core zero-padded wo (cross-batch-group rows zeroed by the host),
    which absorbs the group-dependence the SPMD program cannot express
    (4-core A2A would need mesh support, which requires >4 cores).
  - LN gamma/beta are folded into wq/wk/wv/w1 (and biases) on the host;
    LN stats use a per-group [128,4] batched reciprocal and the
    normalize runs on the ACT engine (scale=rstd, bias=-mean*rstd), so
    the DVE never serializes the per-tile chain.
  - V is computed weight-stationary like Q/K (ap=512) into VT and moved
    to the s-major V_sb layout with per-head [64,512] xbar transposes.
  - Attention is a flat software pipeline across all 16 (head, chunk)
    boundaries: score matmuls run LOOK tiles ahead of the A@V
    accumulation. Diagonal tiles compute only the live query range;
    causal masking is one gpsimd affine_select per diagonal tile
    (zero-fill replaces the mask multiply). Row sums come free from a
    trailing ones-column in V; 1/sum reaches all 64 partitions with a
    gpsimd partition_broadcast. The normalized y tile is DMA'd straight
    into the A2A input buffer (both group halves).
  - wo is streamed slot-by-slot (8 phase-A slots prefetched under
    attention); bo is folded into the host-side residual rows. yf reads
    are split across both HWDGE queues. w1/w2 stream on separate queues
    with prefetch hidden under the A2A#1 window.
"""

import numpy as np

import concourse.bass as bass
import concourse.mybir as mybir
import concourse.tile as tile
from concourse import bacc, masks

F32 = mybir.dt.float32
BF16 = mybir.dt.bfloat16
AF = mybir.ActivationFunctionType
OP = mybir.AluOpType

N_CORES = 8
P = 128
S = 2048          # sequence length
E = 1024          # embed
FF = 4096         # mlp hidden
HL = 4            # local heads
DH = 64           # head dim
CH = HL * DH      # local channels = 256
R = S // 4        # rows per core owned after AllToAll = 512
ET = E // P       # 8 e-tiles
ST = S // P       # 16 s-tiles
FT = FF // P      # 32 f-tiles
RT = R // P       # 4 own-row tiles
EPS = 1e-5
LOOK = 5          # attention score->AV software-pipeline depth
A2A_GROUP = [[0, 1, 2, 3, 4, 5, 6, 7]]
USE_COLLECTIVE = True
FFN_ACT = AF.Gelu


def build_nc():
    nc = bacc.Bacc("TRN2", target_bir_lowering=False, debug=False,
                   num_devices=N_CORES)

    x_b = nc.declare_dram_parameter("x_b", [S, E], BF16, isOutput=False)
    # residual rows with bo pre-added by the host
    xr_f = nc.declare_dram_parameter("xr_f", [R, E], F32, isOutput=False)
    wqkv_b = nc.declare_dram_parameter("wqkv_b", [E, 3 * CH], BF16,
                                       isOutput=False)
    bq_f = nc.declare_dram_parameter("bq_f", [P, 2], F32, isOutput=False)
    bk_f = nc.declare_dram_parameter("bk_f", [P, 2], F32, isOutput=False)
    bv_f = nc.declare_dram_parameter("bv_f", [P, 2], F32, isOutput=False)
    # wo expanded to 2048 rows: row i*256+d*128+p = wo[(i%4)*256+d*128+p]
    # for cores i in this core's batch group, zeros for the other group.
    wo_b = nc.declare_dram_parameter("wo_b", [2 * E, E], BF16, isOutput=False)
    w1_b = nc.declare_dram_parameter("w1_b", [E, FF], BF16, isOutput=False)
    b1_f = nc.declare_dram_parameter("b1_f", [P, FT], F32, isOutput=False)
    w2_b = nc.declare_dram_parameter("w2_b", [FF, E], BF16, isOutput=False)
    b2_b = nc.declare_dram_parameter("b2_b", [E], BF16, isOutput=False)
    out_p = nc.declare_dram_parameter("out", [R, E], F32, isOutput=True)

    with tile.TileContext(nc) as tc:
        _emit(nc, tc, locals())
    nc.compile()
    return nc


def _emit(nc, tc, t):
    x_b, xr_f = t["x_b"], t["xr_f"]
    wqkv_b = t["wqkv_b"]
    bq_f, bk_f, bv_f = t["bq_f"], t["bk_f"], t["bv_f"]
    wo_b = t["wo_b"]
    w1_b, b1_f, w2_b, b2_b = t["w1_b"], t["b1_f"], t["w2_b"], t["b2_b"]
    out_p = t["out_p"]

    const = tc.alloc_tile_pool(name="const", bufs=1)
    wpool = tc.alloc_tile_pool(name="wpool", bufs=1)
    act = tc.alloc_tile_pool(name="act", bufs=1)
    xpool = tc.alloc_tile_pool(name="xpool", bufs=16)
    xnpool = tc.alloc_tile_pool(name="xnpool", bufs=2)
    scrpool = tc.alloc_tile_pool(name="scrpool", bufs=1)
    stats = tc.alloc_tile_pool(name="stats", bufs=6)
    gstats = tc.alloc_tile_pool(name="gstats", bufs=2)
    vtpool = tc.alloc_tile_pool(name="vtpool", bufs=2)
    eppool = tc.alloc_tile_pool(name="eppool", bufs=7)
    recpool = tc.alloc_tile_pool(name="recpool", bufs=2)
    ystpool = tc.alloc_tile_pool(name="ystpool", bufs=2)
    xrpool = tc.alloc_tile_pool(name="xrpool", bufs=4)
    wopool = tc.alloc_tile_pool(name="wopool", bufs=8)
    w1pool = tc.alloc_tile_pool(name="w1pool", bufs=3)
    w2pool = tc.alloc_tile_pool(name="w2pool", bufs=3)
    opool = tc.alloc_tile_pool(name="opool", bufs=2)
    dram = tc.alloc_tile_pool(name="dram", bufs=1, space="DRAM")
    pmm = tc.alloc_tile_pool(name="pmm", bufs=5, space="PSUM")
    pacc = tc.alloc_tile_pool(name="pacc", bufs=2, space="PSUM")
    pnew = tc.alloc_tile_pool(name="pnew", bufs=1, space="PSUM")

    # ---- first x tile rides the sync queue before anything else ----
    x_tiles = {}
    x_tiles[0] = xpool.tile([P, E], BF16, name="xt_t")
    nc.sync.dma_start(out=x_tiles[0][:], in_=x_b[0:P, :])

    # ---- small constants + QKV weights (scalar HWDGE queue) ----
    bqc = const.tile([P, 2], F32, name="bqc")
    nc.scalar.dma_start(out=bqc[:], in_=bq_f[:, :])
    bkc = const.tile([P, 2], F32, name="bkc")
    nc.scalar.dma_start(out=bkc[:], in_=bk_f[:, :])
    bvc = const.tile([P, 2], F32, name="bvc")
    nc.scalar.dma_start(out=bvc[:], in_=bv_f[:, :])
    wqkv_sb = wpool.tile([P, ET, 3 * CH], BF16, name="wqkv_sb")
    nc.scalar.dma_start(out=wqkv_sb[:],
                        in_=wqkv_b.rearrange("(a p) c -> p a c", p=P))
    b1c = const.tile([P, FT], F32, name="b1c")
    nc.scalar.dma_start(out=b1c[:], in_=b1_f[:, :])
    b2r = const.tile([1, E], BF16, name="b2r")
    nc.scalar.dma_start(out=b2r[:], in_=b2_b.rearrange("(a f) -> a f", a=1))

    ones_row = const.tile([1, P], BF16, name="ones_row")
    nc.vector.memset(ones_row[:], 1.0)
    eps_c = const.tile([P, 1], F32, name="eps_c")
    nc.vector.memset(eps_c[:], EPS)

    # ---- persistent activations ----
    # hT and gT share one 32KB/partition slot (hT dies after QKV).
    hT = act.tile([P, ET, S], BF16, tag="big", name="hT")
    QT = act.tile([P, 2, S], BF16, name="QT")      # 2 head-pairs packed
    # KT and h2nT share a slot (KT dies after attention).
    KT = act.tile([P, 2, S], BF16, tag="mid", name="KT")
    # v with a trailing ones column (col 64) so A@V also yields sum(exp);
    # head stride padded to 80 elements -- the xbar transpose needs the
    # destination runs 32-byte aligned
    V_sb = act.tile([P, ST, HL, DH + 16], BF16, name="V_sb")
    h2_sb = act.tile([P, RT, E], F32, name="h2_sb")
    # both yf pairs live in the 32KB "big" slot between hT's death (end of
    # QKV) and gT's birth (FFN1)
    yfb = act.tile([P, 2, 8, R], BF16, tag="big", name="yfb")

    nc.vector.memset(V_sb[:, :, :, DH:DH + 1], 1.0)

    # ========= layernorm: batched stats for a group of up to 4 tiles =========
    # Stats per tile on DVE/ACT; one [128,n] reciprocal per group; the
    # normalize itself runs on ACT (out = x*rstd - mean*rstd).
    def layernorm_group(srcs, dstT, sts):
        n = len(srcs)
        vgrp = gstats.tile([P, 4], F32, tag="vgrp", name="vgrp")
        means = []
        for i, src in enumerate(srcs):
            s_sum = stats.tile([P, 1], F32, tag=f"s_sum{i}", name="s_sum")
            nc.vector.reduce_sum(s_sum[:], src, axis=mybir.AxisListType.X)
            s_mean = stats.tile([P, 1], F32, tag=f"s_mean{i}", name="s_mean")
            nc.scalar.mul(s_mean[:], s_sum[:], 1.0 / E)
            means.append(s_mean)
            scr_t = scrpool.tile([P, E], BF16, name="scr_t")
            s_msq = stats.tile([P, 1], F32, tag=f"s_msq{i}", name="s_msq")
            nc.scalar.activation(scr_t[:], src, AF.Square, scale=1.0 / 32.0,
                                 accum_out=s_msq[:])
            s_mu2 = stats.tile([P, 1], F32, tag=f"s_mu2{i}", name="s_mu2")
            nc.scalar.square(s_mu2[:], s_mean[:])
            nc.vector.tensor_scalar(vgrp[:, i:i + 1], s_msq[:], s_mu2[:],
                                    None, op0=OP.subtract)
        sgrp = gstats.tile([P, 4], F32, tag="sgrp", name="sgrp")
        nc.scalar.activation(sgrp[:, 0:n], vgrp[:, 0:n], AF.Sqrt,
                             bias=eps_c[:])
        rgrp = gstats.tile([P, 4], F32, tag="rgrp", name="rgrp")
        nc.vector.reciprocal(rgrp[:, 0:n], sgrp[:, 0:n])
        for i, src in enumerate(srcs):
            nmr = stats.tile([P, 1], F32, tag=f"nmr{i}", name="nmr")
            nc.vector.tensor_scalar(nmr[:], means[i], rgrp[:, i:i + 1], -1.0,
                                    op0=OP.mult, op1=OP.mult)
            xn_t = xnpool.tile([P, E], BF16, name="xn_t")
            nc.scalar.activation(xn_t[:], src, AF.Identity,
                                 scale=rgrp[:, i:i + 1], bias=nmr[:])
            st = sts[i]
            nc.sync.dma_start_transpose(
                out=dstT[:, :, st * P:(st + 1) * P], in_=xn_t[:])

    # ========= LN1 + QKV + V, in groups of 4 s-tiles =========
    def qkv_group(g):
        qs = slice(g * 512, (g + 1) * 512)
        for d in range(2):
            # Q
            ps = pmm.tile([P, 512], F32, tag="mm", name="ps_q")
            for et in range(ET):
                nc.tensor.matmul(ps[:], lhsT=wqkv_sb[:, et, d * P:(d + 1) * P],
                                 rhs=hT[:, et, qs],
                                 start=(et == 0), stop=(et == ET - 1))
            nc.vector.tensor_scalar(QT[:, d, qs], ps[:], bqc[:, d:d + 1],
                                    None, op0=OP.add)
            # K
            ps = pmm.tile([P, 512], F32, tag="mm", name="ps_k")
            for et in range(ET):
                nc.tensor.matmul(ps[:],
                                 lhsT=wqkv_sb[:, et, CH + d * P:CH + (d + 1) * P],
                                 rhs=hT[:, et, qs],
                                 start=(et == 0), stop=(et == ET - 1))
            nc.scalar.activation(KT[:, d, qs], ps[:], AF.Identity,
                                 bias=bkc[:, d:d + 1])
            # V (weight-stationary, transposed afterwards)
            ps = pmm.tile([P, 512], F32, tag="mm", name="ps_v")
            for et in range(ET):
                nc.tensor.matmul(ps[:],
                                 lhsT=wqkv_sb[:, et, 2 * CH + d * P:2 * CH + (d + 1) * P],
                                 rhs=hT[:, et, qs],
                                 start=(et == 0), stop=(et == ET - 1))
            vt_t = vtpool.tile([P, 512], BF16, name="vt_t")
            nc.vector.tensor_scalar(vt_t[:], ps[:], bvc[:, d:d + 1],
                                    None, op0=OP.add)
            # per-head transpose into s-major V_sb (ones column untouched);
            # all transposes ride the sync queue -- two concurrent xbar
            # transposes on different queues corrupt each other
            for hl in range(2):
                nc.sync.dma_start_transpose(
                    out=V_sb[:, 4 * g:4 * g + 4, 2 * d + hl, 0:DH],
                    in_=vt_t[hl * DH:(hl + 1) * DH, :])

    for g in range(4):
        srcs, sts = [], []
        for st in range(4 * g, 4 * g + 4):
            if st not in x_tiles:
                x_tiles[st] = xpool.tile([P, E], BF16, name="xt_t")
                nc.sync.dma_start(out=x_tiles[st][:],
                                  in_=x_b[st * P:(st + 1) * P, :])
            srcs.append(x_tiles[st][:])
            sts.append(st)
        layernorm_group(srcs, hT, sts)
        qkv_group(g)

    # ========= attention =========
    # A2A buffers: ag_in[d][j] = head-pair d's y^T for dest-quarter j%4.
    ag_in = [dram.tile([8, P, 512], BF16, name=f"ag_in{d}") for d in range(2)]
    ag_out = [dram.tile([8, P, 512], BF16, name=f"ag_out{d}")
              for d in range(2)]

    def send_pair(d):
        if USE_COLLECTIVE:
            nc.gpsimd.collective_compute(
                "AllToAll", OP.bypass, replica_groups=A2A_GROUP,
                ins=[ag_in[d].opt()], outs=[ag_out[d].opt()])
        else:
            nc.sync.dma_start(out=ag_out[d][:, :, :], in_=ag_in[d][:, :, :])

    def flush_chunk(h, qc, acc):
        d, po = h // 2, (h % 2) * DH
        recf = recpool.tile([1, 512], F32, tag="recf", name="recf")
        nc.vector.reciprocal(recf[:], acc[DH:DH + 1, :])
        recb = recpool.tile([1, 512], BF16, tag="recb", name="recb")
        nc.vector.tensor_copy(recb[:], recf[:])
        rbc = recpool.tile([DH, 512], BF16, tag="rbc", name="rbc")
        nc.gpsimd.partition_broadcast(rbc[:], recb[:])
        yst = ystpool.tile([DH, 512], BF16, name="yst")
        nc.vector.tensor_mul(yst[:], acc[0:DH, :], rbc[:])
        nc.sync.dma_start(out=ag_in[d][qc, po:po + DH, :], in_=yst[:])
        nc.sync.dma_start(out=ag_in[d][4 + qc, po:po + DH, :], in_=yst[:])

    # warm the ep pool so the full-width affine_select on the first
    # diagonal tiles never reads uninitialized SBUF
    for _ in range(7):
        ep_w = eppool.tile([P, 512], BF16, name="ep")
        nc.vector.memset(ep_w[:, 0:512], 0.0)

    chunks = [(h, qc) for h in range(HL) for qc in (1, 2, 3, 0)]
    pend = []  # (acc, h, kt, ep, nk, flush_info|None)

    def yf_read(h, eng=None):
        # head h lands at partitions (h%2)*64 of pair h//2
        d, po = h // 2, (h % 2) * DH
        (eng or nc.sync).dma_start(
            out=yfb[po:po + DH, d, :, :],
            in_=ag_out[h].rearrange("i p s -> p i s"))

    def emit_av(job):
        acc, h, kt, ep, nk, fl = job
        nc.tensor.matmul(acc[0:DH + 1, :], lhsT=V_sb[:, kt, h, 0:DH + 1], rhs=ep[:],
                         start=(kt == 0), stop=(kt == nk - 1))
        if fl is not None:
            flush_chunk(fl[0], fl[1], acc)
            if fl == (1, 0):
                send_pair(0)
                post_pair0()
            elif fl == (3, 1):
                yf0_read(0, 3)
            elif fl == (3, 2):
                yf0_read(3, 6)
            elif fl == (3, 3):
                yf0_read(6, 8)
            elif fl == (3, 0):
                send_pair(1)

    wo_tiles = []

    def post_pair0():
        # phase-A wo slots + residual rows + yf0 ride the scalar queue,
        # hidden under the second half of attention
        for i in range(8):
            wo_t = wopool.tile([P, E], BF16, name="wo_t")
            nc.scalar.dma_start(
                out=wo_t[:],
                in_=wo_b[2 * i * P:(2 * i + 1) * P, :])
            wo_tiles.append(wo_t)
        for st in range(RT):
            xr_t = xrpool.tile([P, E], F32, name="xr_t")
            nc.scalar.dma_start(out=xr_t[:], in_=xr_f[st * P:(st + 1) * P, :])
            xr_tiles.append(xr_t)

    xr_tiles = []
    post_pair0()  # no deps; loads land before the A2A#0 window

    for (h, qc) in chunks:
        d, po = h // 2, (h % 2) * DH
        nk = (qc + 1) * 4
        acc = pacc.tile([P, 512], F32, tag="acc", name="acc")
        for kt in range(nk):
            off = max(0, (kt - qc * 4) * P)
            ps = pmm.tile([P, 512], F32, tag="mm", name="ps_s")
            nc.tensor.matmul(ps[:, off:512],
                             lhsT=KT[po:po + DH, d, kt * P:(kt + 1) * P],
                             rhs=QT[po:po + DH, d,
                                    qc * 512 + off:(qc + 1) * 512],
                             start=True, stop=True)
            ep = eppool.tile([P, 512], BF16, name="ep")
            nc.scalar.activation(ep[:, off:512], ps[:, off:512], AF.Exp)
            if kt >= qc * 4:
                # zero query cols < off and the intra-tile upper triangle
                nc.gpsimd.affine_select(
                    out=ep[:], in_=ep[:], compare_op=OP.is_ge, fill=0.0,
                    base=-off, channel_multiplier=-1, pattern=[[1, 512]])
            fl = (h, qc) if kt == nk - 1 else None
            pend.append((acc, h, kt, ep, nk, fl))
            while len(pend) > LOOK:
                emit_av(pend.pop(0))
    while pend:
        emit_av(pend.pop(0))
    while due:
        _, fl2, acc2 = due.pop(0)
        run_flush(fl2, acc2)

    # head-2 half of yf1 (its collective completed mid-attention), then
    # w1 prefetch, then the collective-gated head-3 half
    yf_read(2)
    w1_tiles = []
    for fc in range(2):
        w1_t = w1pool.tile([P, ET, 256], BF16, name="w1_t")
        nc.sync.dma_start(
            out=w1_t[:],
            in_=w1_b[:, fc * 256:(fc + 1) * 256].rearrange(
                "(a p) f -> p a f", p=P))
        w1_tiles.append(w1_t)

    # ========= out-projection =========
    def oacc_tile(idx):
        if idx < 5:
            return pmm.tile([P, 512], F32, tag="mm", name=f"oacc{idx}")
        if idx < 7:
            return pacc.tile([P, 512], F32, tag="acc", name=f"oacc{idx}")
        return pnew.tile([P, 512], F32, tag="o2", name=f"oacc{idx}")

    oaccs = [oacc_tile(i) for i in range(8)]
    for i in range(8):
        for st in range(RT):
            for ec in range(2):
                es = slice(ec * 512, (ec + 1) * 512)
                nc.tensor.matmul(oaccs[st * 2 + ec][:],
                                 lhsT=yfb[:, 0, i, st * P:(st + 1) * P],
                                 rhs=wo_tiles[i][:, es],
                                 start=(i == 0), stop=False)
    # phase-B wo slots + yf1 second half + w2 prefetch on the scalar queue
    for i in range(8):
        wo_t = wopool.tile([P, E], BF16, name="wo_t")
        nc.scalar.dma_start(out=wo_t[:],
                            in_=wo_b[(2 * i + 1) * P:(2 * i + 2) * P, :])
        wo_tiles.append(wo_t)
    yf_read(3, eng=nc.scalar)
    w2_tiles = []
    for fc in range(2):
        w2_t = w2pool.tile([P, E], BF16, name="w2_t")
        nc.scalar.dma_start(out=w2_t[:], in_=w2_b[fc * P:(fc + 1) * P, :])
        w2_tiles.append(w2_t)

    for i in range(8):
        for st in range(RT):
            for ec in range(2):
                es = slice(ec * 512, (ec + 1) * 512)
                nc.tensor.matmul(oaccs[st * 2 + ec][:],
                                 lhsT=yfb[:, 1, i, st * P:(st + 1) * P],
                                 rhs=wo_tiles[8 + i][:, es],
                                 start=False, stop=(i == 7))
    for st in range(RT):
        for ec in range(2):
            es = slice(ec * 512, (ec + 1) * 512)
            nc.vector.tensor_add(h2_sb[:, st, es], oaccs[st * 2 + ec][:],
                                 xr_tiles[st][:, es])

    # ========= LN2 =========
    h2nT = act.tile([P, ET, R], BF16, tag="mid", name="h2nT")
    layernorm_group([h2_sb[:, st, :] for st in range(RT)], h2nT,
                    list(range(RT)))

    # ========= FFN1 (gelu) =========
    gT = act.tile([P, FT, R], BF16, tag="big", name="gT")
    for fc in range(16):
        if fc < 2:
            w1_t = w1_tiles[fc]
        else:
            w1_t = w1pool.tile([P, ET, 256], BF16, name="w1_t")
            nc.sync.dma_start(
                out=w1_t[:],
                in_=w1_b[:, fc * 256:(fc + 1) * 256].rearrange(
                    "(a p) f -> p a f", p=P))
        for ft in range(2):
            ftg = fc * 2 + ft
            ps = pmm.tile([P, 512], F32, tag="mm", name="ps_f1")
            for et in range(ET):
                nc.tensor.matmul(ps[:],
                                 lhsT=w1_t[:, et, ft * P:(ft + 1) * P],
                                 rhs=h2nT[:, et, :],
                                 start=(et == 0), stop=(et == ET - 1))
            nc.scalar.activation(gT[:, ftg, :], ps[:], FFN_ACT,
                                 bias=b1c[:, ftg:ftg + 1])

    # ========= FFN2 + residual =========
    pnew.release()
    pacc.release()
    pmm.release()
    pffn = tc.alloc_tile_pool(name="pffn", bufs=1, space="PSUM")
    accs = [pffn.tile([P, E], F32, name=f"facc{st}") for st in range(RT)]
    for ftg in range(FT):
        if ftg < 2:
            w2_t = w2_tiles[ftg]
        else:
            w2_t = w2pool.tile([P, E], BF16, name="w2_t")
            nc.scalar.dma_start(out=w2_t[:],
                                in_=w2_b[ftg * P:(ftg + 1) * P, :])
        for st in range(RT):
            for ec in range(2):
                es = slice(ec * 512, (ec + 1) * 512)
                nc.tensor.matmul(accs[st][:, es],
                                 lhsT=gT[:, ftg, st * P:(st + 1) * P],
                                 rhs=w2_t[:, es],
                                 start=(ftg == 0), stop=False)
    for st in range(RT):
        for ec in range(2):
            es = slice(ec * 512, (ec + 1) * 512)
            nc.tensor.matmul(accs[st][:, es], lhsT=ones_row[0:1, 0:P],
                             rhs=b2r[0:1, es], start=False, stop=True)
            o_t = opool.tile([P, 512], F32, name="o_t")
            nc.vector.tensor_add(o_t[:], accs[st][:, es], h2_sb[:, st, es])
            eng = nc.sync if ec == 0 else nc.scalar
            eng.dma_start(out=out_p[st * P:(st + 1) * P, es], in_=o_t[:])

    for pool in (pffn, dram, opool, w2pool, w1pool, wopool, xrpool, ystpool,
                 recpool, eppool, vtpool, gstats, stats, scrpool, xnpool,
                 xpool, act, wpool, const):
        pool.release()


_NC_CACHE = None


def _get_nc():
    global _NC_CACHE
    if _NC_CACHE is None:
        _NC_CACHE = build_nc()
    return _NC_CACHE


def kernel(**inputs):
    import ml_dtypes
    bf = ml_dtypes.bfloat16
    nc = _get_nc()

    x = np.asarray(inputs["x"], np.float32)
    wq = np.asarray(inputs["wq"], np.float32)
    wk = np.asarray(inputs["wk"], np.float32)
    wv = np.asarray(inputs["wv"], np.float32)
    wo = np.asarray(inputs["wo"], np.float32)
    w1 = np.asarray(inputs["w1"], np.float32)
    w2 = np.asarray(inputs["w2"], np.float32)
    bq = np.asarray(inputs["bq"], np.float32)
    bk = np.asarray(inputs["bk"], np.float32)
    bv = np.asarray(inputs["bv"], np.float32)
    bo = np.asarray(inputs["bo"], np.float32)
    b1 = np.asarray(inputs["b1"], np.float32)
    b2 = np.asarray(inputs["b2"], np.float32)
    ln1g = np.asarray(inputs["ln1_g"], np.float32)
    ln1b = np.asarray(inputs["ln1_b"], np.float32)
    ln2g = np.asarray(inputs["ln2_g"], np.float32)
    ln2b = np.asarray(inputs["ln2_b"], np.float32)

    sc = 1.0 / np.sqrt(DH)
    # fold LN1 gamma into wq/wk/wv rows and LN1 beta into the biases;
    # same for LN2 gamma/beta into w1/b1. Kernel LN is pure normalize.
    wq_f = ln1g[:, None] * wq * sc
    wk_f = ln1g[:, None] * wk
    wv_f = ln1g[:, None] * wv
    bq_fold = bq * sc + ln1b @ (wq * sc)
    bk_fold = bk + ln1b @ wk
    bv_fold = bv + ln1b @ wv
    w1_f = ln2g[:, None] * w1
    b1_fold = b1 + ln2b @ w1

    in_maps = []
    for core in range(N_CORES):
        b, tp = core // 4, core % 4
        c0 = tp * CH
        wqkv = np.concatenate(
            [wq_f[:, c0:c0 + CH], wk_f[:, c0:c0 + CH], wv_f[:, c0:c0 + CH]],
            axis=1)
        # wo expanded: rows i*256+c hold wo[(i-4b)*256+c] for cores i in
        # this core's batch group, zeros for the other group's rows
        woe = np.zeros((N_CORES, CH, E), np.float32)
        for r_ in range(4):
            woe[4 * b + r_] = wo[r_ * CH:(r_ + 1) * CH]
        in_maps.append({
            "x_b": np.ascontiguousarray(x[b]).astype(bf),
            "xr_f": np.ascontiguousarray(
                x[b, tp * R:(tp + 1) * R] + bo[None, :]),
            "wqkv_b": np.ascontiguousarray(wqkv).astype(bf),
            "bq_f": np.ascontiguousarray(
                bq_fold[c0:c0 + CH].reshape(2, P).T),
            "bk_f": np.ascontiguousarray(
                bk_fold[c0:c0 + CH].reshape(2, P).T),
            "bv_f": np.ascontiguousarray(
                bv_fold[c0:c0 + CH].reshape(2, P).T),
            "wo_b": np.ascontiguousarray(woe.reshape(2 * E, E)).astype(bf),
            "w1_b": np.ascontiguousarray(w1_f).astype(bf),
            "b1_f": np.ascontiguousarray(b1_fold.reshape(FT, P).T),
            "w2_b": w2.astype(bf), "b2_b": b2.astype(bf),
        })

    from concourse.bass_utils import run_bass_kernel_spmd
    import os
    kw = {}
    if os.environ.get("BASS_TRACE"):
        kw = dict(trace=True, trace_cores=list(range(N_CORES)))
    res = run_bass_kernel_spmd(nc, in_maps, core_ids=list(range(N_CORES)), **kw)
    if res.exec_time_ns is not None:
        print(f"HW exec time: {res.exec_time_ns} ns")
        print(f"HW exec time mean: {res.mean_exec_time_ns} ns")

    out = np.empty((2, S, E), np.float32)
    for core in range(N_CORES):
        b, tp = core // 4, core % 4
        out[b, tp * R:(tp + 1) * R] = res.results[core]["out"]
    return out
